# revision 11
# baseline (speedup 1.0000x reference)
"""Trainium2 Bass kernel for nn_MemoryCore (retrieval KNN min-distance).

Problem: embedding [8192, 512], memory_bank [65536, 512] (fp32) ->
patch_scores [8192, 1] = min over the bank of euclidean distance.

Strategy (8 NeuronCores, SPMD):
  - Shard the memory bank (M axis) 8 ways; every core sees all queries.
  - fp8(E4M3) inputs, PE DoubleRow matmuls (256-deep contraction per MM):
    psum[m, q] = (-2*bank) @ emb, psum tiles span 2 banks (1024 queries).
  - Min stage split across DVE and ACT so neither is the bottleneck:
      DVE route: rm = min(psum + m_sq[m], rm)   (one fused STT, bf16 rm)
      ACT route: t = Identity(psum + m_sq[m]) -> bf16 SBUF (scalar engine),
                 rm = min(t, rm)                (2-byte DVE op)
    Two rm chains (even/odd bank tile) decouple the DVE dependency chain.
  - Per-core result: rm [128, 8192] bf16 (128 bank slots x all queries),
    DMA'd to HBM. Host does the cross-partition min, +x_sq, sqrt, and the
    min across the 8 cores.
"""
import numpy as np
import ml_dtypes
import concourse.bacc as bacc
import concourse.mybir as mybir
import concourse.tile as tile
from concourse.bass_utils import run_bass_kernel_spmd

N_CORES = 8
N, M, D = 8192, 65536, 512
MS = M // N_CORES       # 8192 bank rows per core
K4 = D // 128           # 4 fp8 contraction planes of 128
QB = 1024               # query block width (psum tile spans 2 banks)
NB = N // QB            # 8 query blocks
MT = MS // 128          # 64 bank tiles
BIG = 1e30
DT8 = mybir.dt.float8e4
NP8 = ml_dtypes.float8_e4m3

_CACHE = {}


def _build_kernel():
    nc = bacc.Bacc("TRN2", target_bir_lowering=False, debug=False,
                   num_devices=N_CORES)

    embT_d = nc.dram_tensor("embT8", [128, K4 * N], DT8, kind="ExternalInput")
    bankT_d = nc.dram_tensor("bankT8", [128, K4 * MS], DT8, kind="ExternalInput")
    msq_d = nc.dram_tensor("msq", [128, MT], mybir.dt.float32, kind="ExternalInput")
    rm_d = nc.dram_tensor("rm_out", [128, N], mybir.dt.bfloat16,
                          kind="ExternalOutput")

    DR = mybir.MatmulPerfMode.DoubleRow

    with tile.TileContext(nc) as tc:
        with (
            tc.tile_pool(name="persist", bufs=1) as persist,
            tc.tile_pool(name="rmp", bufs=4) as rmp,
            tc.tile_pool(name="tp", bufs=4) as tp,
            tc.tile_pool(name="psum", bufs=4, space="PSUM") as psum,
        ):
            msq = persist.tile([128, MT], mybir.dt.float32, tag="msq")
            nc.sync.dma_start(msq[:], msq_d[:])

            emb_s = persist.tile([128, K4, N], DT8, tag="embs")
            bank_s = persist.tile([128, K4, MS], DT8, tag="banks")
            # chunked loads round-robined over 4 queue engines, ordered so
            # data lands just ahead of consumption (emb block 0, bank tiles
            # in mt order, then the rest of emb)
            qs = [nc.sync, nc.gpsimd, nc.scalar]
            qi = 0

            def q():
                nonlocal qi
                qi += 1
                return qs[qi % 3]

            for k in range(K4):
                q().dma_start(emb_s[:, k, :QB], embT_d[:, k * N:k * N + QB])
            for k in range(K4):
                q().dma_start(bank_s[:, k, :QB], bankT_d[:, k * MS:k * MS + QB])
            for k in range(K4):
                q().dma_start(emb_s[:, k, QB:], embT_d[:, k * N + QB:(k + 1) * N])
            for c in range(1, MS // QB):
                for k in range(K4):
                    q().dma_start(
                        bank_s[:, k, c * QB:(c + 1) * QB],
                        bankT_d[:, k * MS + c * QB:k * MS + (c + 1) * QB])

            for nb in range(NB):
                rm_a = rmp.tile([128, QB], mybir.dt.bfloat16, tag="rma")
                rm_b = rmp.tile([128, QB], mybir.dt.bfloat16, tag="rmb")
                nc.gpsimd.memset(rm_a[:], BIG)
                nc.gpsimd.memset(rm_b[:], BIG)
                for mt in range(MT):
                    rm = rm_a if mt % 2 == 0 else rm_b
                    ps = psum.tile([128, QB], mybir.dt.float32, tag="ps")
                    for kk in range(2):  # each 512-col half is one PSUM bank
                        for h in range(2):
                            nc.tensor.matmul(
                                ps[:, h * 512:(h + 1) * 512],
                                bank_s[:, 2 * kk:2 * kk + 2, mt * 128:(mt + 1) * 128],
                                emb_s[:, 2 * kk:2 * kk + 2,
                                      nb * QB + h * 512:nb * QB + (h + 1) * 512],
                                start=(kk == 0),
                                stop=(kk == 1),
                                perf_mode=DR,
                            )
                    if mt % 4 == 0:
                        # DVE: rm = min(psum + m_sq[m], rm) in one op
                        nc.vector.scalar_tensor_tensor(
                            out=rm[:],
                            in0=ps[:],
                            scalar=msq[:, mt:mt + 1],
                            in1=rm[:],
                            op0=mybir.AluOpType.add,
                            op1=mybir.AluOpType.min,
                        )
                    else:
                        # ACT drains psum (+m_sq bias) to bf16, DVE mins it
                        t = tp.tile([128, QB], mybir.dt.bfloat16, tag="t")
                        nc.scalar.add(t[:], ps[:], msq[:, mt:mt + 1])
                        nc.vector.tensor_tensor(
                            out=rm[:], in0=t[:], in1=rm[:],
                            op=mybir.AluOpType.min)
                nc.vector.tensor_tensor(
                    out=rm_a[:], in0=rm_b[:], in1=rm_a[:],
                    op=mybir.AluOpType.min)
                nc.sync.dma_start(rm_d[:, nb * QB:(nb + 1) * QB], rm_a[:])

    nc.compile()
    return nc


def _pack_kT(mat_T: np.ndarray, width: int) -> np.ndarray:
    """[D, width] fp32 -> [128, K4, width] fp8 with plane k = rows k*128..+128."""
    return np.ascontiguousarray(
        mat_T.reshape(K4, 128, width).transpose(1, 0, 2)).astype(NP8)


def kernel(embedding: np.ndarray, memory_bank: np.ndarray) -> np.ndarray:
    emb = np.asarray(embedding, dtype=np.float32)
    bank = np.asarray(memory_bank, dtype=np.float32)
    assert emb.shape == (N, D) and bank.shape == (M, D)

    if "nc" not in _CACHE:
        _CACHE["nc"] = _build_kernel()
    nc = _CACHE["nc"]

    embT8 = _pack_kT(emb.T, N).reshape(128, K4 * N)
    x_sq = np.einsum("nd,nd->n", emb, emb, dtype=np.float64).astype(np.float32)

    in_maps = []
    for c in range(N_CORES):
        shard = bank[c * MS:(c + 1) * MS]
        bankT8 = _pack_kT((-2.0 * shard).T, MS).reshape(128, K4 * MS)
        m_sq = np.einsum("md,md->m", shard, shard, dtype=np.float64).astype(np.float32)
        msq = np.ascontiguousarray(m_sq.reshape(MS // 128, 128).T)
        in_maps.append({"embT8": embT8, "bankT8": bankT8, "msq": msq})

    _CACHE["last_in_maps"] = in_maps
    try:
        res = run_bass_kernel_spmd(nc, in_maps, core_ids=list(range(N_CORES)))
    except Exception:
        # a previously-wedged NeuronCore reports unrecoverable once and then
        # recovers; one retry clears it
        import time
        time.sleep(2.0)
        res = run_bass_kernel_spmd(nc, in_maps, core_ids=list(range(N_CORES)))

    # gather: each core returns rm [128, N] bf16 = min over its bank tiles of
    # (m_sq - 2 x.m), per (bank slot, query). Min over slots and cores, then
    # + x_sq and sqrt.
    per_core = np.stack([
        np.asarray(res.results[c]["rm_out"], dtype=np.float32).min(axis=0)
        for c in range(N_CORES)
    ])
    dist_sq = np.maximum(per_core.min(axis=0) + x_sq, 0.0)
    return np.sqrt(dist_sq).reshape(N, 1).astype(np.float32)


# revision 16
# speedup vs baseline: 1.0367x; 1.0367x over previous
"""Trainium2 Bass kernel for nn_MemoryCore (retrieval KNN min-distance).

Problem: embedding [8192, 512], memory_bank [65536, 512] (fp32) ->
patch_scores [8192, 1] = min over the bank of euclidean distance.

Strategy (8 NeuronCores, SPMD):
  - Shard the memory bank (M axis) 8 ways; every core sees all queries.
  - fp8(E4M3) inputs, PE DoubleRow matmuls (256-deep contraction per MM):
    psum[m, q] = (-2*bank) @ emb, psum tiles span 2 banks (1024 queries).
  - Min stage split across DVE and ACT so neither is the bottleneck:
      DVE route: rm = min(psum + m_sq[m], rm)   (one fused STT, bf16 rm)
      ACT route: t = Identity(psum + m_sq[m]) -> bf16 SBUF (scalar engine),
                 rm = min(t, rm)                (2-byte DVE op)
    Two rm chains (even/odd bank tile) decouple the DVE dependency chain.
  - Per-core result: rm [128, 8192] bf16 (128 bank slots x all queries),
    DMA'd to HBM. Host does the cross-partition min, +x_sq, sqrt, and the
    min across the 8 cores.
"""
import numpy as np
import ml_dtypes
import concourse.bacc as bacc
import concourse.mybir as mybir
import concourse.tile as tile
from concourse.bass_utils import run_bass_kernel_spmd

N_CORES = 8
N, M, D = 8192, 65536, 512
MS = M // N_CORES       # 8192 bank rows per core
K4 = D // 128           # 4 fp8 contraction planes of 128
QB = 1024               # query block width (psum tile spans 2 banks)
NB = N // QB            # 8 query blocks
MT = MS // 128          # 64 bank tiles
BIG = 1e30
DT8 = mybir.dt.float8e4
NP8 = ml_dtypes.float8_e4m3

_CACHE = {}


def _build_kernel():
    nc = bacc.Bacc("TRN2", target_bir_lowering=False, debug=False,
                   num_devices=N_CORES)

    embT_d = nc.dram_tensor("embT8", [128, K4 * N], DT8, kind="ExternalInput")
    bankT_d = nc.dram_tensor("bankT8", [128, K4 * MS], DT8, kind="ExternalInput")
    msq_d = nc.dram_tensor("msq", [128, MT], mybir.dt.float32, kind="ExternalInput")
    rm_d = nc.dram_tensor("rm_out", [128, N], mybir.dt.bfloat16,
                          kind="ExternalOutput")

    DR = mybir.MatmulPerfMode.DoubleRow

    with tile.TileContext(nc) as tc:
        with (
            tc.tile_pool(name="persist", bufs=1) as persist,
            tc.tile_pool(name="tp", bufs=4) as tp,
            tc.tile_pool(name="psum", bufs=4, space="PSUM") as psum,
        ):
            msq = persist.tile([128, MT], mybir.dt.float32, tag="msq")
            nc.scalar.dma_start(msq[:], msq_d[:])

            emb_s = persist.tile([128, K4, N], DT8, tag="embs")
            bank_s = persist.tile([128, K4, MS], DT8, tag="banks")
            # Engine streams are in-order, so keep each queue free for what it
            # must do early: scalar gets only the tiny early loads (its ACT
            # drains start ~20us in), sync takes the rest of emb + rm stores,
            # gpsimd interleaves bank chunks (mt order) with the rm memsets.
            for k in range(2):
                nc.scalar.dma_start(emb_s[:, k, :QB], embT_d[:, k * N:k * N + QB])
            for k in range(2, K4):
                nc.sync.dma_start(emb_s[:, k, :QB], embT_d[:, k * N:k * N + QB])
            for c in range(1, N // QB):
                for k in range(K4):
                    nc.sync.dma_start(
                        emb_s[:, k, c * QB:(c + 1) * QB],
                        embT_d[:, k * N + c * QB:k * N + (c + 1) * QB])

            rms = []
            for nb in range(NB):
                rm_a = persist.tile([128, QB], mybir.dt.bfloat16, tag=f"rma{nb}")
                rm_b = persist.tile([128, QB], mybir.dt.bfloat16, tag=f"rmb{nb}")
                rms.append((rm_a, rm_b))
            for c in range(MS // QB):
                for k in range(K4):
                    nc.gpsimd.dma_start(
                        bank_s[:, k, c * QB:(c + 1) * QB],
                        bankT_d[:, k * MS + c * QB:k * MS + (c + 1) * QB])
                if c < NB:
                    nc.gpsimd.memset(rms[c][0][:], BIG)
                    nc.gpsimd.memset(rms[c][1][:], BIG)

            for nb in range(NB):
                rm_a, rm_b = rms[nb]
                for mt in range(MT):
                    rm = rm_a if mt % 2 == 0 else rm_b
                    ps = psum.tile([128, QB], mybir.dt.float32, tag="ps")
                    for kk in range(2):  # each 512-col half is one PSUM bank
                        for h in range(2):
                            nc.tensor.matmul(
                                ps[:, h * 512:(h + 1) * 512],
                                bank_s[:, 2 * kk:2 * kk + 2, mt * 128:(mt + 1) * 128],
                                emb_s[:, 2 * kk:2 * kk + 2,
                                      nb * QB + h * 512:nb * QB + (h + 1) * 512],
                                start=(kk == 0),
                                stop=(kk == 1),
                                perf_mode=DR,
                            )
                    if mt % 4 == 0:
                        # DVE: rm = min(psum + m_sq[m], rm) in one op
                        nc.vector.scalar_tensor_tensor(
                            out=rm[:],
                            in0=ps[:],
                            scalar=msq[:, mt:mt + 1],
                            in1=rm[:],
                            op0=mybir.AluOpType.add,
                            op1=mybir.AluOpType.min,
                        )
                    else:
                        # ACT drains psum (+m_sq bias) to bf16, DVE mins it
                        t = tp.tile([128, QB], mybir.dt.bfloat16, tag="t")
                        nc.scalar.add(t[:], ps[:], msq[:, mt:mt + 1])
                        nc.vector.tensor_tensor(
                            out=rm[:], in0=t[:], in1=rm[:],
                            op=mybir.AluOpType.min)
                nc.vector.tensor_tensor(
                    out=rm_a[:], in0=rm_b[:], in1=rm_a[:],
                    op=mybir.AluOpType.min)
                nc.sync.dma_start(rm_d[:, nb * QB:(nb + 1) * QB], rm_a[:])

    nc.compile()
    return nc


def _pack_kT(mat_T: np.ndarray, width: int) -> np.ndarray:
    """[D, width] fp32 -> [128, K4, width] fp8 with plane k = rows k*128..+128."""
    return np.ascontiguousarray(
        mat_T.reshape(K4, 128, width).transpose(1, 0, 2)).astype(NP8)


def kernel(embedding: np.ndarray, memory_bank: np.ndarray) -> np.ndarray:
    emb = np.asarray(embedding, dtype=np.float32)
    bank = np.asarray(memory_bank, dtype=np.float32)
    assert emb.shape == (N, D) and bank.shape == (M, D)

    if "nc" not in _CACHE:
        _CACHE["nc"] = _build_kernel()
    nc = _CACHE["nc"]

    embT8 = _pack_kT(emb.T, N).reshape(128, K4 * N)
    x_sq = np.einsum("nd,nd->n", emb, emb, dtype=np.float64).astype(np.float32)

    in_maps = []
    for c in range(N_CORES):
        shard = bank[c * MS:(c + 1) * MS]
        bankT8 = _pack_kT((-2.0 * shard).T, MS).reshape(128, K4 * MS)
        m_sq = np.einsum("md,md->m", shard, shard, dtype=np.float64).astype(np.float32)
        msq = np.ascontiguousarray(m_sq.reshape(MS // 128, 128).T)
        in_maps.append({"embT8": embT8, "bankT8": bankT8, "msq": msq})

    _CACHE["last_in_maps"] = in_maps
    try:
        res = run_bass_kernel_spmd(nc, in_maps, core_ids=list(range(N_CORES)))
    except Exception:
        # a previously-wedged NeuronCore reports unrecoverable once and then
        # recovers; one retry clears it
        import time
        time.sleep(2.0)
        res = run_bass_kernel_spmd(nc, in_maps, core_ids=list(range(N_CORES)))

    # gather: each core returns rm [128, N] bf16 = min over its bank tiles of
    # (m_sq - 2 x.m), per (bank slot, query). Min over slots and cores, then
    # + x_sq and sqrt.
    per_core = np.stack([
        np.asarray(res.results[c]["rm_out"], dtype=np.float32).min(axis=0)
        for c in range(N_CORES)
    ])
    dist_sq = np.maximum(per_core.min(axis=0) + x_sq, 0.0)
    return np.sqrt(dist_sq).reshape(N, 1).astype(np.float32)
